# revision 34
# baseline (speedup 1.0000x reference)
"""Trainium2 Bass kernel for nn_ECA_69544110457542.

Math (per row r=(b,t)):
  dyn   = x[:, :31] @ Wd + bd
  value = x[:, 31] * Wv[0] + bv
  xhn   = [dyn | human@Wh+bh | nature@Wn+bn]                      (768 ch)
  pre_j = sum_k cw[t,k] * xhn[perm[ainv[j]+k-3]] + conv_b[t]      (j<256)
  sel   = softmax(relu(pre))
  out   = 0.5*(dyn*sel) @ Wvd1  +  0.5*dyn @ Wvd1 + value @ Wvd2 + bvd
          `------ device ------'  `------- folded into wdf (host) -----'

Sharding is T-MAJOR data parallel: core c takes all 256 batches for the 8
timesteps t in [8c, 8c+8); each 128-row tile holds one fixed t.  Since the
whole conv+shuffle is linear in the activations and cw[t,k] is host-known,
the host folds the 5 conv taps AND the cw[t,:] scaling into a single per-t
gathered weight matrix  Wfold_t = sum_k cw[t,k] * Wg_k  [193, 256], so the
logits need just 2 accumulating matmuls per tile (K split 128+65) instead
of 10, and no on-device activation pre-scaling at all.  conv_b[t] rides the
exp bias; relu runs after exp as max(exp,1) (exact: exp(relu(x)) =
max(exp(x),1)) whose DVE accum_out is the softmax denominator.

Six-stage pipeline over 16 tiles (tile j, iteration it):
  S1 @ j    PE : dyn_j (256 cols), pre_j (2x256, accumulated)
  S2 @ j+1  ACT: exp(pre_j) bias=conv_b[t];  DVE: max+accum, reciprocal;
             gate z = exm*sinv*dyn  (alternating DVE / Pool)
  S3 @ j+3  PE : fold_j = b0 @ wfold (start of out-PSUM), transpose z;
             Pool: drain zT from PSUM to SBUF
  S4 @ j+4  PE : zT @ 0.5*Wvd1 accumulated into fold_j PSUM (stop)
  S5 @ j+5  ACT/Pool (alternating): drain out tile into pair buffer;
             SP: one HWDGE DMA per pair of tiles

PSUM: pdy/pre/pfo [P,256]x3 + ptz bf16 x3 = ~5.25 banks.  Bacc.finalize()
splits any multi-wait instruction into EventSemaphore preludes (1-wait ISA
limit).
"""

import sys

sys.path.insert(0, "/opt/trn_rl_repo")

from contextlib import ExitStack

import ml_dtypes
import numpy as np

import concourse.bacc as bacc
import concourse.bass as bass
import concourse.tile as tile
from concourse import mybir
from concourse.bass_utils import run_bass_kernel_spmd

# problem constants
B, T, E = 256, 64, 256
XS, DS = 32, 31
HT, NT_ = 80, 80
C = 3 * E
KW = 5
NCORES = 8
TPC = T // NCORES          # 8 timesteps per core
R = B * TPC                # 2048 rows per core (t-major: r = t*256 + b)
P = 128
NTILES = R // P            # 16 (two batch-halves per t)
AK = XS + 1 + HT + NT_     # 193 act rows: x(32) | ones | h(80) | n(80)
K2 = AK - 128              # 65

# packed-constants layout, fp32 slot offsets in [128, WPACK], ordered by
# first-use time so the input DMAs can stream them just in time
O_WDF = 0                   # bf16 [128, 512]: wdyn | wfold (rows 33: zero)
O_WF0 = 256                 # bf16 [128, 512]: t0 [wf1_t0 256 | wf2_t0 256]
O_CB = 512                  # fp32 [128, 8]: conv_b[t] per local t
O_IDB = 520                 # bf16 identity [128, 128]
O_WV1 = 584                 # bf16 [128, 512]: 0.5*Wvd1 rows 0:128 | 128:256
O_WFR = 840                 # bf16 [128, 7, 512]: t1..t7 folded weights
WPACK = O_WFR + (TPC - 1) * 256   # 2632
ABCH = 8                    # actb DMA chunks (256 rows each, 1KB lines)

F32 = mybir.dt.float32
BF16 = mybir.dt.bfloat16
MULT = mybir.AluOpType.mult
ADD = mybir.AluOpType.add
EXP = mybir.ActivationFunctionType.Exp

_NC_CACHE = None
LAST_RESULTS = None
TRACE = False


def _patched_drain_and_barrier(self, tick_clock, wait_clock):
    # The stock kernel-tail drain puts every processor's final-tick wait on a
    # single Drain instruction; this walrus build rejects multi-wait
    # instructions, so spread the waits over a chain of drains instead.
    import bass_rust as _br
    from concourse.vector_clock import ScopedClock

    nc = self.nc
    drain_inst = nc.sync.drain()
    wait_clock.add_sem_waits(
        drain_inst.ins, ScopedClock({None: tick_clock.global_clock})
    )
    si = drain_inst.ins.sync_info
    if si is not None and len(si.on_wait) > 1:
        waits = list(si.on_wait)
        drain_inst.ins.sync_info = _br.SyncInfo(
            on_wait=[waits[0]], on_update=list(si.on_update)
        )
        for w in waits[1:]:
            d2 = nc.sync.drain()
            d2.ins.sync_info = _br.SyncInfo(on_wait=[w], on_update=[])
    nc.all_engine_barrier()
    assert self.sems is not None
    popped = nc._tile_sem_poison_stack.pop()
    assert popped is self._sem_poison
    if SEM_CLEAR:
        nc.clear_and_free_semaphores(list(self.sems.allocated().values()))
        nc.all_engine_barrier()
    else:
        # Single-TileContext kernel: nothing after this context reuses the
        # sems, so skip the per-semaphore clear storm (~6us across engines)
        # and just reset the DMA queue state.  NOTE: leaves sems nonzero at
        # NEFF exit — only valid if each execution re-initializes them
        # (verified by the double-execution check in test.py).
        nc.gpsimd.dma_reset(range(min(s.num for s in self.sems.allocated().values()),
                                  max(s.num for s in self.sems.allocated().values()) + 1))
        nc.all_engine_barrier()


SEM_CLEAR = False
SEM_POOL = 96
tile.TileContext._drain_and_barrier = _patched_drain_and_barrier


def _build_nc():
    # Bacc (not plain Bass): its finalize() runs the passes that split
    # multi-wait instructions into EventSemaphore preludes.
    nc = bacc.Bacc()
    # Shrink the declared semaphore pool: walrus zeroes every declared sem at
    # NEFF exit (~50 serialized EVENT_SEMAPHOREs per engine, ~5.5us of
    # teardown).  Keep sems already allocated by Bacc init (engine barrier
    # sems) out of the rebuilt free list.
    full = nc._kernel_sem_range
    lim = min(full.start + SEM_POOL, full.stop)
    nc._kernel_sem_range = range(full.start, lim)
    free_now = [s for s in nc._state.free_semaphores if s < lim]
    nc._state.reset_free_semaphores(free_now)
    actb_d = nc.dram_tensor("actb", [P, ABCH * 2 * 256], BF16, kind="ExternalInput")
    wpack_d = nc.dram_tensor("wpack", [P, WPACK], F32, kind="ExternalInput")
    out_d = nc.dram_tensor("out", [R, E], F32, kind="ExternalOutput")
    # chunk-major activations: chunk c = rows [256c, 256c+256), contiguous
    # 1KB lines per partition per chunk
    actb4 = actb_d[:, :].rearrange("p (c two r) -> p c two r", c=ABCH, two=2)
    # per-pair output view: rows = pair*256 + half*128 + p
    out_pair = out_d[:, :].rearrange("(q two p) e -> q p two e", two=2, p=P)

    with tile.TileContext(nc) as tc, ExitStack() as ctx:
        consts = ctx.enter_context(tc.tile_pool(name="consts", bufs=1))
        # PSUM pair tiles: pd [P,2,512] (2 banks) x2, pfo [P,2,256] x2,
        # ptz [P,4,128] bf16 x2  -> exactly 8 banks
        psD = ctx.enter_context(tc.tile_pool(name="psD", bufs=2, space="PSUM"))
        psF = ctx.enter_context(tc.tile_pool(name="psF", bufs=2, space="PSUM"))
        psT = ctx.enter_context(tc.tile_pool(name="psT", bufs=2, space="PSUM"))
        pc = ctx.enter_context(tc.tile_pool(name="pc", bufs=4))
        pz = ctx.enter_context(tc.tile_pool(name="pz", bufs=4))
        pzT = ctx.enter_context(tc.tile_pool(name="pzT", bufs=4))
        po = ctx.enter_context(tc.tile_pool(name="po", bufs=4))
        psm = ctx.enter_context(tc.tile_pool(name="psm", bufs=4))

        wp = consts.tile([P, WPACK], F32)
        ab = consts.tile([P, ABCH, 2, 256], BF16)

        # input DMAs, finest-first so the first tiles' operands land ASAP.
        # SP and ACT HWDGE queues transfer concurrently (~60 GB/s each).
        nc.sync.dma_start(ab[:, 0], actb4[:, 0])
        nc.scalar.dma_start(wp[:, O_WDF:O_WF0], wpack_d[:, O_WDF:O_WF0])
        nc.sync.dma_start(wp[:, O_WF0:O_CB], wpack_d[:, O_WF0:O_CB])
        nc.sync.dma_start(wp[:, O_CB:O_WFR], wpack_d[:, O_CB:O_WFR])
        nc.scalar.dma_start(wp[:, O_WFR : O_WFR + 256],
                            wpack_d[:, O_WFR : O_WFR + 256])
        for c in range(1, ABCH):
            nc.sync.dma_start(ab[:, c], actb4[:, c])
        nc.scalar.dma_start(wp[:, O_WFR + 256 : O_WFR + 768],
                            wpack_d[:, O_WFR + 256 : O_WFR + 768])
        nc.scalar.dma_start(wp[:, O_WFR + 768 : O_WFR + 1280],
                            wpack_d[:, O_WFR + 768 : O_WFR + 1280])
        nc.scalar.dma_start(wp[:, O_WFR + 1280 : WPACK],
                            wpack_d[:, O_WFR + 1280 : WPACK])

        wpb = wp[:].bitcast(BF16)
        wdyn = wpb[:, 2 * O_WDF : 2 * O_WDF + 256]
        wfold = wpb[:, 2 * O_WDF + 256 : 2 * O_WDF + 512]
        wv1a = wpb[:, 2 * O_WV1 : 2 * O_WV1 + E]
        wv1b = wpb[:, 2 * O_WV1 + E : 2 * O_WV1 + 2 * E]
        identb = wpb[:, 2 * O_IDB : 2 * O_IDB + P]
        cb = wp[:, O_CB : O_CB + TPC]

        def wf_t(t):
            base = 2 * O_WF0 if t == 0 else 2 * (O_WFR + (t - 1) * 256)
            return wpb[:, base : base + 256], wpb[0:K2, base + 256 : base + 512]

        def brow(i, half):
            # activation block for tile i (128 rows), K-split half
            c, off = divmod(i, 2)
            lim = P if half == 0 else K2
            return ab[0:lim, c, half, off * P : (off + 1) * P]

        # warmup: each compute engine observes the first weights DMA once so
        # the first pipelined op doesn't carry the DMA waits (Bacc splits any
        # tile-0 multi-waits into EventSemaphores).  The ACT warmup runs EXP
        # so the activation table loads during the DMA shadow, off the
        # critical path.
        at = psm.tile([P, 1], F32, tag="wm")
        nc.scalar.activation(at[:], wp[:, 0:1], func=EXP)
        dt_ = psm.tile([P, 1], F32, tag="wm2")
        nc.vector.tensor_copy(dt_[:], wp[:, 0:1])
        gt_ = psm.tile([P, 1], F32, tag="wm3")
        nc.gpsimd.tensor_copy(gt_[:], wp[:, 0:1])

        st1 = {}   # pair m -> pd pair tile [P, 2, 512] = [dyn | pre] x2
        stz = {}   # pair m -> (z pair, sinv pair)
        st3 = {}   # pair m -> (ptz pair, pfo pair)
        st4 = {}   # pair m -> (zTs pair, pfo pair)
        st5 = {}   # pair m -> pfo pair

        # per-pair schedule (pair m, tiles 2m / 2m+1):
        #   S1  @ it=2m, 2m+1 : PE dyn+pre per tile into the pd pair
        #   S2  @ it=2m+2     : ACT exp-pair + dyn-copy-pair; DVE max x2,
        #                       recip-pair, gate x2 (all SBUF)
        #   S3  @ it=2m+3/2m+4: PE fold per tile + transposes per tile;
        #                       DVE zTs pair copy at 2m+4
        #   S4  @ it=2m+5     : PE s2b x2 into the pfo pair (stop)
        #   S5  @ it=2m+6     : ACT pair drain; SP pair DMA
        for it in range(NTILES + 7):
            # ---- S4 PE (pair (it-5)//2): zT @ 0.5*Wvd1 accum into fold ---
            if it >= 5 and (it - 5) % 2 == 0 and (m4 := (it - 5) // 2) < NTILES // 2:
                zTs4, pfo4 = st4.pop(m4)
                for h in range(2):
                    nc.tensor.matmul(pfo4[:, h, :], zTs4[:, 2 * h, :], wv1a,
                                     start=False, stop=False,
                                     skip_group_check=True)
                    nc.tensor.matmul(pfo4[:, h, :], zTs4[:, 2 * h + 1, :], wv1b,
                                     start=False, stop=True,
                                     skip_group_check=True)
                st5[m4] = pfo4

            # ---- S3 PE (tile it-3): fold matmul + transpose z ------------
            if 3 <= it < NTILES + 3:
                j3 = it - 3
                m3, h3 = divmod(j3, 2)
                if h3 == 0:
                    pfo = psF.tile([P, 2, E], F32, tag="pfo", name=f"pfo{m3}")
                    ptz = psT.tile([P, 4, P], BF16, tag="ptz", name=f"ptz{m3}")
                    zp, _ = stz[m3]
                    st3[m3] = (ptz, pfo, zp)
                ptz, pfo, zp = st3[m3]
                # start=True only on the pair's first fold: start re-arms
                # pending-zero for the WHOLE bank, so a second start=True
                # would wipe the first fold's partial sum.  The h=1 region is
                # still pending from h=0's start, so its opening write
                # zero-fills either way.
                nc.tensor.matmul(pfo[:, h3, :], brow(j3, 0), wfold,
                                 start=(h3 == 0), stop=False,
                                 skip_group_check=True)
                nc.tensor.transpose(ptz[:, 2 * h3, :], zp[:, h3, 0:128], identb)
                nc.tensor.transpose(ptz[:, 2 * h3 + 1, :], zp[:, h3, 128:256],
                                    identb)
                if h3 == 1:
                    ptz, pfo, _ = st3.pop(m3)
                    stz.pop(m3)
                    zTs = pzT.tile([P, 4, P], BF16, tag="zTs", name=f"zTs{m3}")
                    nc.vector.tensor_copy(zTs[:], ptz[:])
                    st4[m3] = (zTs, pfo)

            # ---- S1 PE (tile it): dyn + logits ---------------------------
            if it < NTILES:
                i = it
                t = i // 2
                h = i % 2
                b0 = brow(i, 0)
                wf1t, wf2t = wf_t(t)
                if h == 0:
                    st1[t] = psD.tile([P, 2, 2 * E], F32, tag="pd", name=f"pd{t}")
                pd = st1[t]
                nc.tensor.matmul(pd[:, h, 0:E], b0, wdyn,
                                 start=True, stop=True, skip_group_check=True)
                nc.tensor.matmul(pd[:, h, E : 2 * E], b0, wf1t,
                                 start=True, stop=False, skip_group_check=True)
                nc.tensor.matmul(pd[:, h, E : 2 * E], brow(i, 1), wf2t,
                                 start=False, stop=True, skip_group_check=True)

            # ---- S2 (pair it//2 - 1): softmax + gate, paired -------------
            if it >= 2 and it % 2 == 0 and (m2 := it // 2 - 1) < NTILES // 2:
                pdm = st1.pop(m2)
                exp_ = pc.tile([P, 2, E], BF16, tag="exp", name=f"exp{m2}")
                nc.scalar.activation(exp_[:], pdm[:, :, E : 2 * E], func=EXP,
                                     bias=cb[:, m2 : m2 + 1])
                dynb = pc.tile([P, 2, E], BF16, tag="dynb", name=f"dynb{m2}")
                nc.scalar.copy(dynb[:], pdm[:, :, 0:E])
                exm = pc.tile([P, 2, E], BF16, tag="exm", name=f"exm{m2}")
                ssum = psm.tile([P, 2], F32, tag="ssum", name=f"ssum{m2}")
                for h in range(2):
                    nc.vector.tensor_scalar(exm[:, h, :], exp_[:, h, :], 1.0,
                                            0.0, op0=mybir.AluOpType.max,
                                            op1=mybir.AluOpType.add,
                                            accum_out=ssum[:, h : h + 1])
                sinv = psm.tile([P, 2], F32, tag="sinv", name=f"sinv{m2}")
                nc.vector.reciprocal(sinv[:], ssum[:])
                zp = pz.tile([P, 2, E], BF16, tag="z", name=f"z{m2}")
                for h in range(2):
                    nc.vector.scalar_tensor_tensor(zp[:, h, :], exm[:, h, :],
                                                   sinv[:, h : h + 1],
                                                   dynb[:, h, :],
                                                   op0=MULT, op1=MULT)
                stz[m2] = (zp, sinv)

            # ---- S5 (pair (it-6)//2): out drain + pair DMA ---------------
            if it >= 6 and it % 2 == 0 and (m5 := (it - 6) // 2) < NTILES // 2:
                pfo5 = st5.pop(m5)
                obp = po.tile([P, 2, E], F32, tag="obp", name=f"obp{m5}")
                nc.scalar.copy(obp[:], pfo5[:])
                nc.sync.dma_start(out_pair[m5], obp[:])

    nc.finalize()
    return nc


def _host_prep(x, human, nature, perm, Wv, bv, Wd, bd, Wh, bh, Wn, bn,
               conv_w, conv_b, Wvd, bvd):
    f = np.float32
    bf = ml_dtypes.bfloat16
    x = np.asarray(x, f)
    human = np.asarray(human, f)
    nature = np.asarray(nature, f)
    Wv = np.asarray(Wv, f); bv = np.asarray(bv, f)
    Wd = np.asarray(Wd, f); bd = np.asarray(bd, f)
    Wh = np.asarray(Wh, f); bh = np.asarray(bh, f)
    Wn = np.asarray(Wn, f); bn = np.asarray(bn, f)
    conv_w = np.asarray(conv_w, f)
    conv_b = np.asarray(conv_b, f)
    Wvd = np.asarray(Wvd, f); bvd = np.asarray(bvd, f)
    perm = np.asarray(perm).astype(np.int64)

    Wvd1 = Wvd[:E, :]
    Wvd2 = Wvd[E:, :]

    # t-major activation rows: r = t_global*B + b
    acts = np.concatenate(
        [
            x.reshape(B * T, XS),
            np.ones((B * T, 1), f),
            human.reshape(B * T, HT),
            nature.reshape(B * T, NT_),
        ],
        axis=1,
    )
    acts_tm = np.ascontiguousarray(
        acts.reshape(B, T, AK).transpose(1, 0, 2).reshape(B * T, AK)
    )
    actsT = np.ascontiguousarray(acts_tm.T)  # [193, T*B]
    # chunk-major per core: [P, ABCH, 2, 256] with chunk c = rows [256c, 256c+256)
    actb = np.zeros((P, NCORES, ABCH, 2, 256), bf)
    a3 = actsT.reshape(AK, NCORES, ABCH, 256)
    actb[:, :, :, 0, :] = a3[0:128]
    actb[0:K2, :, :, 1, :] = a3[128:AK]

    # gathered conv weight taps Wg_k [AK, E] (fp32, folded per-t below)
    ainv = np.argsort(perm)
    Wg = np.zeros((KW, AK, E), f)
    for k in range(KW):
        pos = ainv[:E] + k - 3
        for j in range(E):
            pj = pos[j]
            if 0 <= pj < C:
                c = perm[pj]
                if c < E:
                    Wg[k, 0:DS, j] = Wd[:, c]
                    Wg[k, 32, j] = bd[c]
                elif c < 2 * E:
                    Wg[k, 33:113, j] = Wh[:, c - E]
                    Wg[k, 32, j] = bh[c - E]
                else:
                    Wg[k, 113:193, j] = Wn[:, c - 2 * E]
                    Wg[k, 32, j] = bn[c - 2 * E]
    # per-t fold: Wfold_t = sum_k cw[t,k] * Wg_k   [T, AK, E]
    cw = conv_w[:, 0, :]                             # [T, KW]
    wfold_t = np.einsum("tk,kae->tae", cw, Wg)       # fp32

    # dyn | folded-linear weights (rows 33:128 zero so the matmul can use
    # the full 128-row stationary block)
    wdf = np.zeros((128, 512), f)
    wdf[0:DS, 0:E] = Wd
    wdf[32, 0:E] = bd
    wdf[0:DS, E:512] = 0.5 * (Wd @ Wvd1)
    wdf[31, E:512] = Wv[0] @ Wvd2
    wdf[32, E:512] = 0.5 * (bd @ Wvd1) + bv @ Wvd2 + bvd

    wpack = np.zeros((NCORES, P, WPACK), f)
    wpv = wpack.view(bf)  # bf16 alias [NCORES, 128, 2*WPACK]
    wv1 = (0.5 * Wvd1).astype(bf)
    for ci in range(NCORES):
        wpv[ci, :, 2 * O_WDF : 2 * O_WDF + 512] = wdf.astype(bf)
        wpv[ci, :, 2 * O_WV1 : 2 * O_WV1 + E] = wv1[0:128]
        wpv[ci, :, 2 * O_WV1 + E : 2 * O_WV1 + 2 * E] = wv1[128:256]
        wpv[ci, :, 2 * O_IDB : 2 * O_IDB + P] = np.eye(P, dtype=bf)
        wpack[ci, :, O_CB : O_CB + TPC] = conv_b[ci * TPC : (ci + 1) * TPC][None, :]
        for tl in range(TPC):
            wt = wfold_t[ci * TPC + tl].astype(bf)   # [AK, E]
            base = 2 * O_WF0 if tl == 0 else 2 * (O_WFR + (tl - 1) * 256)
            wpv[ci, :, base : base + 256] = wt[0:128]
            wpv[ci, 0:K2, base + 256 : base + 512] = wt[128:AK]
    return actb, wpack


def kernel(**inputs):
    global _NC_CACHE, LAST_RESULTS
    actb, wpack = _host_prep(**inputs)

    if _NC_CACHE is None:
        _NC_CACHE = _build_nc()
    nc = _NC_CACHE

    in_maps = []
    for ci in range(NCORES):
        sb = np.ascontiguousarray(actb[:, ci]).reshape(P, ABCH * 2 * 256)
        in_maps.append({"actb": sb, "wpack": wpack[ci]})

    res = run_bass_kernel_spmd(nc, in_maps, core_ids=list(range(NCORES)), trace=TRACE)
    LAST_RESULTS = res

    # de-shard: core c's rows are (t_local, b) for t_global = c*TPC + t_local
    out = np.empty((B, T, E), np.float32)
    for ci in range(NCORES):
        blk = res.results[ci]["out"].reshape(TPC, B, E)
        out[:, ci * TPC : (ci + 1) * TPC, :] = blk.transpose(1, 0, 2)
    return out
